# revision 16
# baseline (speedup 1.0000x reference)
"""Trainium2 Bass kernel for nn_AbsolutePE_LM (single-head causal transformer block + LM head).

Model (fp32 reference):
    h = embed[x] + pe[:C]
    Q = h Wq^T ; K = h Wk^T ; V = h Wv^T
    A = softmax(QK^T/sqrt(D) + causal)
    hidden = h + A V
    logits = hidden Wo^T + bo

Algebraic restructure (device computes fewer FLOPs; host precomputes M):
    M := Wq^T Wk                (host, fp32)
    scores = (h_q M) h_kv^T / sqrt(D)      -> no K projection on device
    attn   = (A h_kv) Wv^T                 -> no V projection on device
    hidden = h_q + attn
    logits = hidden Wo^T       (+ bo on host)

Sharding: 8 cores = (batch b in 0..3) x (query-set s in {A,B}).  The
16 query tiles of 256 rows are split causally-balanced: set A owns
tiles {0,7,2,5}, set B owns {1,6,3,4}, so both core types need the
same per-position key-block bound BOUND=[4,16,8,14] (42 blocks, vs 56
for a contiguous halves split).  Blocks below MUL_FROM=[0,12,4,8] are
causally full for BOTH sets, so the exp() result skips the mask
multiply.  One program on all cores; per-core behaviour is carried by
the input data (gathered rows, causal masks).

Schedule: attention runs query-tile-major (j-major) so hidden rows
j0/j1 finish early; Phase D then starts on row-half m0..3 for the
first KA vocab groups (re-streaming those Wo tiles once more for the
m4..7 catch-up pass) while the j2/j3 attention tail hides under the
Phase D matmul stream.  All SBUF tiles coexist; PSUM pools are
managed manually to fit the 8 banks per phase.  Input DMAs are
ordered along the attention critical path (Q' inputs, keys, masks,
kv-rows, Wv, residual halves, then the Wo stream).  Vector+GpSimd
split the Q' evictions and residual adds; Scalar does Exp and all
Phase D psum evictions.

Precision: attention matmuls run fp8(e4m3)+DoubleRow at 2x PE rate
(attn is ~2% of hidden and softmax is near-uniform); residual h and
the vocab projection in bf16; logits emitted bf16 and upcast on host.
Scales are powers of two folded into host-side tensors: embed/pe
carry 2^10, M/Wv^T carry 2^11, Wo^T carries 2^-10 so logits come out
of the last matmul unscaled.
"""

import numpy as np

V, D, MAXLEN, B, C = 32000, 1024, 2048, 4, 2048
P = 128
DH = D // P            # 8 partition tiles over the model dim
NQ = C // 2            # 1024 query rows per core
TQ = NQ // P           # 8 query row-tiles
TKV = C // P           # 16 kv row-tiles
QT = 256               # attention query-tile width
NQT = NQ // QT         # 4 attention query tiles
NKB = C // P           # 16 key blocks of 128
VT = 512               # vocab tile width
N_CORES = 8
KA = 4                 # vocab groups run split m0..3 / m4..7 to hide attention

# causally-balanced query-tile assignment (tiles of 256 rows); the two
# small-bound positions come first so their attention output unblocks
# Phase D with only half the kv data staged
TILES_A = [0, 2, 7, 5]
TILES_B = [1, 3, 6, 4]
BOUND = [2 * max(a, b) + 2 for a, b in zip(TILES_A, TILES_B)]     # [4,16,8,14]
MUL_FROM = [2 * min(a, b) for a, b in zip(TILES_A, TILES_B)]      # [0,12,4,8]
NMASKP = sum((bo - mf) // 2 for bo, mf in zip(BOUND, MUL_FROM))   # 9 pair-masks

SH = 1024.0            # 2^10 scale on h (embed/pe, host)
SW = 2048.0            # 2^11 scale on M and Wv^T (host)

_COMPILED = None


def _build_program():
    import concourse.bacc as bacc
    import concourse.mybir as mybir
    import concourse.tile as tile

    f32 = mybir.dt.float32
    bf16 = mybir.dt.bfloat16
    fp8 = mybir.dt.float8e4
    Exp = mybir.ActivationFunctionType.Exp
    Copy = mybir.ActivationFunctionType.Copy
    DR = mybir.MatmulPerfMode.DoubleRow

    nc = bacc.Bacc("TRN2", target_bir_lowering=False, debug=False, num_devices=N_CORES)

    hqT8_d = nc.dram_tensor("hqT8", [D, NQ], fp8, kind="ExternalInput").ap()
    wm8_d = nc.dram_tensor("wm8", [D, D], fp8, kind="ExternalInput").ap()
    hqT_d = nc.dram_tensor("hqT", [D, NQ], bf16, kind="ExternalInput").ap()
    hkvT8_d = nc.dram_tensor("hkvT8", [D, C], fp8, kind="ExternalInput").ap()
    hkv8_d = nc.dram_tensor("hkv8", [C, D], fp8, kind="ExternalInput").ap()
    wv_d = nc.dram_tensor("wv", [D, D], fp8, kind="ExternalInput").ap()
    woT_d = nc.dram_tensor("woT", [D, V], bf16, kind="ExternalInput").ap()
    mask_d = nc.dram_tensor("mask", [P, NMASKP, 2, QT], fp8, kind="ExternalInput").ap()
    y_d = nc.dram_tensor("y", [NQ, V], bf16, kind="ExternalOutput").ap()

    hqT8_r = hqT8_d.rearrange("(dh p) q -> p dh q", p=P)
    wm8_r = wm8_d.rearrange("(dh p) e -> p dh e", p=P)
    hqT_r = hqT_d.rearrange("(dh p) q -> p dh q", p=P)
    hkvT8_r = hkvT8_d.rearrange("(dh p) k -> p dh k", p=P)
    hkv8_r = hkv8_d.rearrange("(t p) e -> p t e", p=P)
    wv_r = wv_d.rearrange("(dh p) e -> p dh e", p=P)
    woT_r = woT_d.rearrange("(dh p) v -> p dh v", p=P)

    with tile.TileContext(nc) as tc:
        with tc.tile_pool(name="persist", bufs=1) as persist, \
             tc.tile_pool(name="att_sb", bufs=6) as att_sb, \
             tc.tile_pool(name="recip_sb", bufs=4) as recip_sb, \
             tc.tile_pool(name="expm_pool", bufs=4) as expm_pool, \
             tc.tile_pool(name="zt_pool", bufs=2) as zt_pool, \
             tc.tile_pool(name="wo_pool", bufs=4) as wo_pool, \
             tc.tile_pool(name="out_sb", bufs=4) as out_sb:
            ones8 = persist.tile([P, 2, P], fp8, tag="ones8")
            nc.gpsimd.memset(ones8[:], 1.0)

            wm8 = persist.tile([P, DH, D], fp8, tag="wm8")
            hqT8 = persist.tile([P, DH, NQ], fp8, tag="hqT8")
            hqT = persist.tile([P, DH, NQ], bf16, tag="hqT")  # becomes hiddenT
            hkvT8 = persist.tile([P, DH, C], fp8, tag="hkvT8")
            h_kv8 = persist.tile([P, TKV, D], fp8, tag="h_kv8")
            QTs8 = persist.tile([P, DH, NQ], fp8, tag="QTs8")
            wv_sb = persist.tile([P, DH, D], fp8, tag="wv_sb")
            mask_sb = persist.tile([P, NMASKP, 2, QT], fp8, tag="mask")

            # staging order = attention critical path: Q' inputs, then the
            # first-half keys/kv/residual (enough for the small-bound query
            # pair j0/j1), then the rest, which retires under Phase D
            kb1 = BOUND[1]   # j0/j1 need key blocks < BOUND[1] only
            nc.sync.dma_start(wm8[:, :, 0:D // 2], wm8_r[:, :, 0:D // 2])
            nc.sync.dma_start(hqT8[:, :, 0:NQ // 2], hqT8_r[:, :, 0:NQ // 2])
            nc.sync.dma_start(wm8[:, :, D // 2:D], wm8_r[:, :, D // 2:D])
            nc.sync.dma_start(hqT8[:, :, NQ // 2:NQ], hqT8_r[:, :, NQ // 2:NQ])
            nc.sync.dma_start(hkvT8[:, :, 0:kb1 * P], hkvT8_r[:, :, 0:kb1 * P])
            nc.sync.dma_start(mask_sb[:], mask_d[:])
            nc.sync.dma_start(wv_sb[:], wv_r[:])
            nc.sync.dma_start(h_kv8[:, 0:kb1, :], hkv8_r[:, 0:kb1, :])
            nc.sync.dma_start(hqT[:, :, 0:2 * QT], hqT_r[:, :, 0:2 * QT])
            nc.sync.dma_start(hkvT8[:, :, kb1 * P:C], hkvT8_r[:, :, kb1 * P:C])
            nc.sync.dma_start(h_kv8[:, kb1:TKV, :], hkv8_r[:, kb1:TKV, :])
            nc.sync.dma_start(hqT[:, :, 2 * QT:NQ], hqT_r[:, :, 2 * QT:NQ])

            # ---- Phase B: Q' = h_q M (fp8 DoubleRow) ----
            # half-outer: Q' columns 0:512 (query tiles j0/j1) complete on
            # the first 1MB of staged input, so scores start early
            with tc.tile_pool(name="qp_ps", bufs=4, space="PSUM") as qp_ps:
                for half in range(2):
                    for eh in range(DH):
                        ps = qp_ps.tile([P, 512], f32, tag="ps")
                        for dhp in range(0, DH, 2):
                            nc.tensor.matmul(
                                ps[:],
                                lhsT=wm8[:, dhp:dhp + 2, eh * P:(eh + 1) * P],
                                rhs=hqT8[:, dhp:dhp + 2, half * 512:(half + 1) * 512],
                                start=(dhp == 0), stop=(dhp == DH - 2),
                                perf_mode=DR,
                            )
                        # ps carries 2^21 (h 2^10 * M 2^11); QTs8 carries 2^11
                        if eh % 2 == 1:
                            nc.scalar.activation(
                                QTs8[:, eh, half * 512:(half + 1) * 512], ps[:],
                                Copy, scale=float(2.0 ** -10))
                        else:
                            nc.vector.tensor_scalar_mul(
                                QTs8[:, eh, half * 512:(half + 1) * 512], ps[:],
                                float(2.0 ** -10))

            # ---- attention, query-tile-major ----
            dn_ps = tc.alloc_tile_pool(name="dn_ps", bufs=1, space="PSUM")
            sc_ps = tc.alloc_tile_pool(name="sc_ps", bufs=2, space="PSUM")
            den = dn_ps.tile([P, NQT, QT], f32, tag="den")
            expm = []
            recips = []
            for _qt in range(NQT):
                expm_t = expm_pool.tile([P, NKB, QT], fp8, tag="expm")
                expm.append(expm_t)
                recip_t = recip_sb.tile([P, QT], f32, tag="recip")
                recips.append(recip_t)
            mask_order = []   # host must build masks in this order
            for j in range(NQT):
                for kbp in range(MUL_FROM[j], BOUND[j], 2):
                    mask_order.append((j, kbp))

            def scores_j(j):
                qs = slice(j * QT, (j + 1) * QT)
                for kbp in range(0, BOUND[j], 2):
                    s_ps = sc_ps.tile([P, 2, QT], f32, tag="sc")
                    for kb in (kbp, kbp + 1):
                        for dhp in range(0, DH, 2):
                            nc.tensor.matmul(
                                s_ps[:, kb - kbp, :],
                                lhsT=hkvT8[:, dhp:dhp + 2, kb * P:(kb + 1) * P],
                                rhs=QTs8[:, dhp:dhp + 2, qs],
                                start=(dhp == 0), stop=(dhp == DH - 2),
                                perf_mode=DR,
                            )
                    # scores carry 2^21 (h 2^10 * Q' 2^11)
                    if kbp >= MUL_FROM[j]:
                        expT = att_sb.tile([P, 2, QT], fp8, tag="expT")
                        nc.scalar.activation(
                            expT[:], s_ps[:], Exp,
                            scale=float(2.0 ** -21 / np.sqrt(D)))
                        mp = mask_order.index((j, kbp))
                        nc.vector.tensor_mul(
                            expm[j][:, kbp:kbp + 2, :], expT[:],
                            mask_sb[:, mp, :, :])
                    else:
                        nc.scalar.activation(
                            expm[j][:, kbp:kbp + 2, :], s_ps[:], Exp,
                            scale=float(2.0 ** -21 / np.sqrt(D)))
                for kbp in range(0, BOUND[j], 2):
                    nc.tensor.matmul(
                        den[:, j, :],
                        lhsT=ones8[:],
                        rhs=expm[j][:, kbp:kbp + 2, :],
                        start=(kbp == 0), stop=(kbp == BOUND[j] - 2),
                        perf_mode=DR,
                    )
                nc.vector.reciprocal(recips[j][:], den[:, j, :])

            def ctail_pair(jlo, zt_ps, at_ps):
                # Z for j=jlo, jlo+1 (separate causal bounds), then one
                # 512-wide projection + residual pass over the row pair
                ZT8 = zt_pool.tile([P, DH, 2 * QT], fp8, tag="ZT8")
                for j in (jlo, jlo + 1):
                    kbm = BOUND[j]
                    for eh in range(DH):
                        z_ps = zt_ps.tile([P, QT], f32, tag="z")
                        for kb2 in range(0, kbm, 2):
                            nc.tensor.matmul(
                                z_ps[:],
                                lhsT=h_kv8[:, kb2:kb2 + 2, eh * P:(eh + 1) * P],
                                rhs=expm[j][:, kb2:kb2 + 2, :],
                                start=(kb2 == 0), stop=(kb2 == kbm - 2),
                                perf_mode=DR,
                            )
                        nc.vector.tensor_mul(
                            ZT8[:, eh, (j - jlo) * QT:(j - jlo + 1) * QT],
                            z_ps[:], recips[j][:])
                # attn_out^T = Wv Z^T, accumulated into hiddenT (2^10)
                qs = slice(jlo * QT, (jlo + 2) * QT)
                for eh in range(DH):
                    a_ps = at_ps.tile([P, 2 * QT], f32, tag="at")
                    for dhp in range(0, DH, 2):
                        nc.tensor.matmul(
                            a_ps[:],
                            lhsT=wv_sb[:, dhp:dhp + 2, eh * P:(eh + 1) * P],
                            rhs=ZT8[:, dhp:dhp + 2, :],
                            start=(dhp == 0), stop=(dhp == DH - 2),
                            perf_mode=DR,
                        )
                    tmp = att_sb.tile([P, 2 * QT], bf16, tag="tmp")
                    nc.scalar.activation(tmp[:], a_ps[:], Copy,
                                         scale=float(2.0 ** -11))
                    nc.gpsimd.tensor_add(hqT[:, eh, qs], hqT[:, eh, qs], tmp[:])

            scores_j(0)
            scores_j(1)
            zt_ps = tc.alloc_tile_pool(name="zt_ps", bufs=2, space="PSUM", side="right")
            at_ps = tc.alloc_tile_pool(name="at_ps", bufs=2, space="PSUM", side="right")
            ctail_pair(0, zt_ps, at_ps)
            scores_j(2)
            scores_j(3)
            sc_ps.release()
            dn_ps.release()

            # ---- Phase D: logits = hiddenT^T WoT (bias added on host) ----
            nt = (V + VT - 1) // VT
            groups = []
            i = 0
            while i < nt:
                n0 = i * VT
                if i + 1 < nt:
                    groups.append([(n0, min(VT, V - n0)), (n0 + VT, min(VT, V - n0 - VT))])
                    i += 2
                else:
                    groups.append([(n0, min(VT, V - n0))])
                    i += 1

            # jobs: (group, m_lo, m_hi); first KA groups run split so the
            # m0..3 half starts as soon as j0/j1 hidden rows are ready
            jobs = [(g, 0, TQ // 2) for g in groups[:KA]] \
                 + [(g, TQ // 2, TQ) for g in groups[:KA]] \
                 + [(g, 0, TQ) for g in groups[KA:]]

            def load_wo(grp):
                gw = sum(nw for _, nw in grp)
                g0 = grp[0][0]
                wo_c0 = wo_pool.tile([P, DH // 2, 2 * VT], bf16, tag="wo")
                wo_c1 = wo_pool.tile([P, DH // 2, 2 * VT], bf16, tag="wo")
                nc.sync.dma_start(wo_c0[:, :, :gw], woT_r[:, 0:DH // 2, g0:g0 + gw])
                nc.sync.dma_start(wo_c1[:, :, :gw], woT_r[:, DH // 2:DH, g0:g0 + gw])
                return wo_c0, wo_c1

            out_ps = tc.alloc_tile_pool(name="out_ps1", bufs=4, space="PSUM")
            wo_cur = load_wo(jobs[0][0])
            for ji, (grp, m_lo, m_hi) in enumerate(jobs):
                gw = sum(nw for _, nw in grp)
                g0 = grp[0][0]
                wo_c0, wo_c1 = wo_cur
                if ji + 1 < len(jobs):
                    wo_cur = load_wo(jobs[ji + 1][0])
                if ji == 2 * KA:
                    # all split jobs done: attention tail is retired, swap
                    # to the full-width psum pool
                    at_ps.release()
                    zt_ps.release()
                    out_ps.release()
                    out_ps = tc.alloc_tile_pool(name="out_ps2", bufs=8, space="PSUM")
                for m in range(m_lo, m_hi):
                    pss = []
                    for _j in grp:
                        ps_t = out_ps.tile([P, VT], f32, tag="out")
                        pss.append(ps_t)
                    for dh in range(DH):
                        wo_t = wo_c0 if dh < DH // 2 else wo_c1
                        for j, (n0, nw) in enumerate(grp):
                            nc.tensor.matmul(
                                pss[j][:, :nw],
                                lhsT=hqT[:, dh, m * P:(m + 1) * P],
                                rhs=wo_t[:, dh % (DH // 2), j * VT:j * VT + nw],
                                start=(dh == 0), stop=(dh == DH - 1),
                            )
                    lo = out_sb.tile([P, 2 * VT], bf16, tag="lo")
                    for j, (n0, nw) in enumerate(grp):
                        nc.scalar.activation(
                            lo[:, j * VT:j * VT + nw], pss[j][:, :nw],
                            Copy, scale=1.0)
                    nc.sync.dma_start(y_d[m * P:(m + 1) * P, g0:g0 + gw], lo[:, :gw])
                # attention tail interleaves with the first split jobs
                if ji == 0:
                    ctail_pair(2, zt_ps, at_ps)
            out_ps.release()

    nc.compile()
    return nc


def _get_program():
    global _COMPILED
    if _COMPILED is None:
        _COMPILED = _build_program()
    return _COMPILED


def _core_rows(hh):
    tiles = TILES_A if hh == 0 else TILES_B
    return np.concatenate([np.arange(t * QT, (t + 1) * QT) for t in tiles])


def kernel(x, embed, pe, Wq, Wk, Wv, Wo, bo):
    import ml_dtypes
    from concourse.bass_utils import run_bass_kernel_spmd

    bf16 = ml_dtypes.bfloat16
    fp8 = ml_dtypes.float8_e4m3fn
    x = np.asarray(x).astype(np.int32)
    embed = np.asarray(embed, dtype=np.float32)
    pe = np.asarray(pe, dtype=np.float32)
    Wq = np.asarray(Wq, dtype=np.float32)
    Wk = np.asarray(Wk, dtype=np.float32)

    h_all = (embed[x.reshape(-1)].reshape(B, C, D) + pe[None, :C, :]) * SH
    h8_all = np.clip(h_all, -240.0, 240.0).astype(fp8)
    h_all = h_all.astype(bf16)
    wm8 = np.ascontiguousarray(
        np.clip((Wq.T @ Wk) * SW, -240.0, 240.0).astype(fp8))
    wv8 = np.ascontiguousarray(
        np.clip(np.asarray(Wv, dtype=np.float32).T * SW, -240.0, 240.0).astype(fp8))
    woT = np.ascontiguousarray(
        (np.asarray(Wo, dtype=np.float32).T * (1.0 / SH)).astype(bf16))
    bo_np = np.asarray(bo, dtype=np.float32).reshape(V)

    nc = _get_program()

    rows = {hh: _core_rows(hh) for hh in range(2)}
    in_maps = []
    for c in range(N_CORES):
        b, hh = c // 2, c % 2
        tiles = TILES_A if hh == 0 else TILES_B
        # pair-masks in program order: j-major, masked pairs only
        blocks = []
        for j in range(NQT):
            for kbp in range(MUL_FROM[j], BOUND[j], 2):
                qpos = tiles[j] * QT + np.arange(QT)
                kpos0 = kbp * P + np.arange(P)
                kpos1 = (kbp + 1) * P + np.arange(P)
                m0 = (qpos[None, :] >= kpos0[:, None]).astype(fp8)
                m1 = (qpos[None, :] >= kpos1[:, None]).astype(fp8)
                blocks.append(np.stack([m0, m1], axis=1))  # [P, 2, QT]
        maskarr = np.ascontiguousarray(np.stack(blocks, axis=1))  # [P, NMASKP, 2, QT]
        assert maskarr.shape == (P, NMASKP, 2, QT)
        hq = h_all[b][rows[hh]]          # [NQ, D] bf16
        hq8 = h8_all[b][rows[hh]]        # [NQ, D] fp8
        in_maps.append({
            "hqT8": np.ascontiguousarray(hq8.T),
            "wm8": wm8,
            "hqT": np.ascontiguousarray(hq.T),
            "hkvT8": np.ascontiguousarray(h8_all[b].T),
            "hkv8": h8_all[b],
            "wv": wv8,
            "woT": woT,
            "mask": maskarr,
        })

    global _last_in_maps
    _last_in_maps = in_maps
    res = run_bass_kernel_spmd(nc, in_maps, core_ids=list(range(N_CORES)))

    out = np.empty((B, C, V), dtype=np.float32)
    for c in range(N_CORES):
        b, hh = c // 2, c % 2
        out[b, rows[hh], :] = res.results[c]["y"].astype(np.float32)
    if np.any(bo_np):
        out += bo_np[None, None, :]
    return out


# revision 19
# speedup vs baseline: 1.0046x; 1.0046x over previous
"""Trainium2 Bass kernel for nn_AbsolutePE_LM (single-head causal transformer block + LM head).

Model (fp32 reference):
    h = embed[x] + pe[:C]
    Q = h Wq^T ; K = h Wk^T ; V = h Wv^T
    A = softmax(QK^T/sqrt(D) + causal)
    hidden = h + A V
    logits = hidden Wo^T + bo

Algebraic restructure (device computes fewer FLOPs; host precomputes M):
    M := Wq^T Wk                (host, fp32)
    scores = (h_q M) h_kv^T / sqrt(D)      -> no K projection on device
    attn   = (A h_kv) Wv^T                 -> no V projection on device
    hidden = h_q + attn
    logits = hidden Wo^T       (+ bo on host)

Sharding: 8 cores = (batch b in 0..3) x (query-set s in {A,B}).  The
16 query tiles of 256 rows are split causally-balanced: set A owns
tiles {0,7,2,5}, set B owns {1,6,3,4}, so both core types need the
same per-position key-block bound BOUND=[4,16,8,14] (42 blocks, vs 56
for a contiguous halves split).  Blocks below MUL_FROM=[0,12,4,8] are
causally full for BOTH sets, so the exp() result skips the mask
multiply.  One program on all cores; per-core behaviour is carried by
the input data (gathered rows, causal masks).

Schedule: attention runs query-tile-major (j-major) so hidden rows
j0/j1 finish early; Phase D then starts on row-half m0..3 for the
first KA vocab groups (re-streaming those Wo tiles once more for the
m4..7 catch-up pass) while the j2/j3 attention tail hides under the
Phase D matmul stream.  All SBUF tiles coexist; PSUM pools are
managed manually to fit the 8 banks per phase.  Input DMAs are
ordered along the attention critical path (Q' inputs, keys, masks,
kv-rows, Wv, residual halves, then the Wo stream).  Vector+GpSimd
split the Q' evictions and residual adds; Scalar does Exp and all
Phase D psum evictions.

Precision: attention matmuls run fp8(e4m3)+DoubleRow at 2x PE rate
(attn is ~2% of hidden and softmax is near-uniform); residual h and
the vocab projection in bf16; logits emitted bf16 and upcast on host.
Scales are powers of two folded into host-side tensors: embed/pe
carry 2^10, M/Wv^T carry 2^11, Wo^T carries 2^-10 so logits come out
of the last matmul unscaled.
"""

import numpy as np

V, D, MAXLEN, B, C = 32000, 1024, 2048, 4, 2048
P = 128
DH = D // P            # 8 partition tiles over the model dim
NQ = C // 2            # 1024 query rows per core
TQ = NQ // P           # 8 query row-tiles
TKV = C // P           # 16 kv row-tiles
QT = 256               # attention query-tile width
NQT = NQ // QT         # 4 attention query tiles
NKB = C // P           # 16 key blocks of 128
VT = 512               # vocab tile width
N_CORES = 8
KA = 4                 # vocab groups run split m0..3 / m4..7 to hide attention

# causally-balanced query-tile assignment (tiles of 256 rows); the two
# small-bound positions come first so their attention output unblocks
# Phase D with only half the kv data staged
TILES_A = [0, 2, 7, 5]
TILES_B = [1, 3, 6, 4]
BOUND = [2 * max(a, b) + 2 for a, b in zip(TILES_A, TILES_B)]     # [4,16,8,14]
MUL_FROM = [2 * min(a, b) for a, b in zip(TILES_A, TILES_B)]      # [0,12,4,8]
NMASKP = sum((bo - mf) // 2 for bo, mf in zip(BOUND, MUL_FROM))   # 9 pair-masks

SH = 1024.0            # 2^10 scale on h (embed/pe, host)
SW = 2048.0            # 2^11 scale on M and Wv^T (host)

_COMPILED = None


def _build_program():
    import concourse.bacc as bacc
    import concourse.mybir as mybir
    import concourse.tile as tile

    f32 = mybir.dt.float32
    bf16 = mybir.dt.bfloat16
    fp8 = mybir.dt.float8e4
    Exp = mybir.ActivationFunctionType.Exp
    Copy = mybir.ActivationFunctionType.Copy
    DR = mybir.MatmulPerfMode.DoubleRow

    nc = bacc.Bacc("TRN2", target_bir_lowering=False, debug=False, num_devices=N_CORES)

    hqT8_d = nc.dram_tensor("hqT8", [D, NQ], fp8, kind="ExternalInput").ap()
    wm8_d = nc.dram_tensor("wm8", [D, D], fp8, kind="ExternalInput").ap()
    hqT_d = nc.dram_tensor("hqT", [D, NQ], bf16, kind="ExternalInput").ap()
    hkvT8_d = nc.dram_tensor("hkvT8", [D, C], fp8, kind="ExternalInput").ap()
    hkv8_d = nc.dram_tensor("hkv8", [C, D], fp8, kind="ExternalInput").ap()
    wv_d = nc.dram_tensor("wv", [D, D], fp8, kind="ExternalInput").ap()
    woT_d = nc.dram_tensor("woT", [D, V], bf16, kind="ExternalInput").ap()
    mask_d = nc.dram_tensor("mask", [P, NMASKP, 2, QT], fp8, kind="ExternalInput").ap()
    y_d = nc.dram_tensor("y", [NQ, V], bf16, kind="ExternalOutput").ap()

    hqT8_r = hqT8_d.rearrange("(dh p) q -> p dh q", p=P)
    wm8_r = wm8_d.rearrange("(dh p) e -> p dh e", p=P)
    hqT_r = hqT_d.rearrange("(dh p) q -> p dh q", p=P)
    hkvT8_r = hkvT8_d.rearrange("(dh p) k -> p dh k", p=P)
    hkv8_r = hkv8_d.rearrange("(t p) e -> p t e", p=P)
    wv_r = wv_d.rearrange("(dh p) e -> p dh e", p=P)
    woT_r = woT_d.rearrange("(dh p) v -> p dh v", p=P)

    with tile.TileContext(nc) as tc:
        with tc.tile_pool(name="persist", bufs=1) as persist, \
             tc.tile_pool(name="att_sb", bufs=6) as att_sb, \
             tc.tile_pool(name="recip_sb", bufs=4) as recip_sb, \
             tc.tile_pool(name="expm_pool", bufs=4) as expm_pool, \
             tc.tile_pool(name="zt_pool", bufs=2) as zt_pool, \
             tc.tile_pool(name="wo_pool", bufs=6) as wo_pool, \
             tc.tile_pool(name="out_sb", bufs=4) as out_sb:
            ones8 = persist.tile([P, 2, P], fp8, tag="ones8")
            nc.gpsimd.memset(ones8[:], 1.0)

            wm8 = persist.tile([P, DH, D], fp8, tag="wm8")
            hqT8 = persist.tile([P, DH, NQ], fp8, tag="hqT8")
            hqT = persist.tile([P, DH, NQ], bf16, tag="hqT")  # becomes hiddenT
            hkvT8 = persist.tile([P, DH, C], fp8, tag="hkvT8")
            h_kv8 = persist.tile([P, TKV, D], fp8, tag="h_kv8")
            QTs8 = persist.tile([P, DH, NQ], fp8, tag="QTs8")
            wv_sb = persist.tile([P, DH, D], fp8, tag="wv_sb")
            mask_sb = persist.tile([P, NMASKP, 2, QT], fp8, tag="mask")

            # staging order = attention critical path: Q' inputs, then the
            # first-half keys/kv/residual (enough for the small-bound query
            # pair j0/j1), then the rest, which retires under Phase D
            kb1 = BOUND[1]   # j0/j1 need key blocks < BOUND[1] only
            nc.sync.dma_start(wm8[:, :, 0:D // 2], wm8_r[:, :, 0:D // 2])
            nc.sync.dma_start(hqT8[:, :, 0:NQ // 2], hqT8_r[:, :, 0:NQ // 2])
            nc.sync.dma_start(wm8[:, :, D // 2:D], wm8_r[:, :, D // 2:D])
            nc.sync.dma_start(hqT8[:, :, NQ // 2:NQ], hqT8_r[:, :, NQ // 2:NQ])
            nc.sync.dma_start(hkvT8[:, :, 0:kb1 * P], hkvT8_r[:, :, 0:kb1 * P])
            nc.sync.dma_start(mask_sb[:], mask_d[:])
            nc.sync.dma_start(wv_sb[:], wv_r[:])
            nc.sync.dma_start(h_kv8[:, 0:kb1, :], hkv8_r[:, 0:kb1, :])
            nc.sync.dma_start(hqT[:, :, 0:2 * QT], hqT_r[:, :, 0:2 * QT])
            nc.sync.dma_start(hkvT8[:, :, kb1 * P:C], hkvT8_r[:, :, kb1 * P:C])
            nc.sync.dma_start(h_kv8[:, kb1:TKV, :], hkv8_r[:, kb1:TKV, :])
            nc.sync.dma_start(hqT[:, :, 2 * QT:NQ], hqT_r[:, :, 2 * QT:NQ])

            # ---- Phase B: Q' = h_q M (fp8 DoubleRow) ----
            # half-outer: Q' columns 0:512 (query tiles j0/j1) complete on
            # the first 1MB of staged input, so scores start early
            with tc.tile_pool(name="qp_ps", bufs=4, space="PSUM") as qp_ps:
                for half in range(2):
                    for eh in range(DH):
                        ps = qp_ps.tile([P, 512], f32, tag="ps")
                        for dhp in range(0, DH, 2):
                            nc.tensor.matmul(
                                ps[:],
                                lhsT=wm8[:, dhp:dhp + 2, eh * P:(eh + 1) * P],
                                rhs=hqT8[:, dhp:dhp + 2, half * 512:(half + 1) * 512],
                                start=(dhp == 0), stop=(dhp == DH - 2),
                                perf_mode=DR,
                            )
                        # ps carries 2^21 (h 2^10 * M 2^11); QTs8 carries 2^11
                        if eh % 2 == 1:
                            nc.scalar.activation(
                                QTs8[:, eh, half * 512:(half + 1) * 512], ps[:],
                                Copy, scale=float(2.0 ** -10))
                        else:
                            nc.vector.tensor_scalar_mul(
                                QTs8[:, eh, half * 512:(half + 1) * 512], ps[:],
                                float(2.0 ** -10))

            # ---- attention, query-tile-major ----
            dn_ps = tc.alloc_tile_pool(name="dn_ps", bufs=1, space="PSUM")
            sc_ps = tc.alloc_tile_pool(name="sc_ps", bufs=2, space="PSUM")
            den = dn_ps.tile([P, NQT, QT], f32, tag="den")
            expm = []
            recips = []
            for _qt in range(NQT):
                expm_t = expm_pool.tile([P, NKB, QT], fp8, tag="expm")
                expm.append(expm_t)
                recip_t = recip_sb.tile([P, QT], f32, tag="recip")
                recips.append(recip_t)
            mask_order = []   # host must build masks in this order
            for j in range(NQT):
                for kbp in range(MUL_FROM[j], BOUND[j], 2):
                    mask_order.append((j, kbp))

            def scores_j(j):
                qs = slice(j * QT, (j + 1) * QT)
                for kbp in range(0, BOUND[j], 2):
                    s_ps = sc_ps.tile([P, 2, QT], f32, tag="sc")
                    for kb in (kbp, kbp + 1):
                        for dhp in range(0, DH, 2):
                            nc.tensor.matmul(
                                s_ps[:, kb - kbp, :],
                                lhsT=hkvT8[:, dhp:dhp + 2, kb * P:(kb + 1) * P],
                                rhs=QTs8[:, dhp:dhp + 2, qs],
                                start=(dhp == 0), stop=(dhp == DH - 2),
                                perf_mode=DR,
                            )
                    # scores carry 2^21 (h 2^10 * Q' 2^11)
                    if kbp >= MUL_FROM[j]:
                        expT = att_sb.tile([P, 2, QT], fp8, tag="expT")
                        nc.scalar.activation(
                            expT[:], s_ps[:], Exp,
                            scale=float(2.0 ** -21 / np.sqrt(D)))
                        mp = mask_order.index((j, kbp))
                        nc.vector.tensor_mul(
                            expm[j][:, kbp:kbp + 2, :], expT[:],
                            mask_sb[:, mp, :, :])
                    else:
                        nc.scalar.activation(
                            expm[j][:, kbp:kbp + 2, :], s_ps[:], Exp,
                            scale=float(2.0 ** -21 / np.sqrt(D)))
                for kbp in range(0, BOUND[j], 2):
                    nc.tensor.matmul(
                        den[:, j, :],
                        lhsT=ones8[:],
                        rhs=expm[j][:, kbp:kbp + 2, :],
                        start=(kbp == 0), stop=(kbp == BOUND[j] - 2),
                        perf_mode=DR,
                    )
                nc.vector.reciprocal(recips[j][:], den[:, j, :])

            def ctail_pair(jlo, zt_ps, at_ps):
                # Z for j=jlo, jlo+1 (separate causal bounds), then one
                # 512-wide projection + residual pass over the row pair
                ZT8 = zt_pool.tile([P, DH, 2 * QT], fp8, tag="ZT8")
                for j in (jlo, jlo + 1):
                    kbm = BOUND[j]
                    for eh in range(DH):
                        z_ps = zt_ps.tile([P, QT], f32, tag="z")
                        for kb2 in range(0, kbm, 2):
                            nc.tensor.matmul(
                                z_ps[:],
                                lhsT=h_kv8[:, kb2:kb2 + 2, eh * P:(eh + 1) * P],
                                rhs=expm[j][:, kb2:kb2 + 2, :],
                                start=(kb2 == 0), stop=(kb2 == kbm - 2),
                                perf_mode=DR,
                            )
                        nc.vector.tensor_mul(
                            ZT8[:, eh, (j - jlo) * QT:(j - jlo + 1) * QT],
                            z_ps[:], recips[j][:])
                # attn_out^T = Wv Z^T, accumulated into hiddenT (2^10)
                qs = slice(jlo * QT, (jlo + 2) * QT)
                for eh in range(DH):
                    a_ps = at_ps.tile([P, 2 * QT], f32, tag="at")
                    for dhp in range(0, DH, 2):
                        nc.tensor.matmul(
                            a_ps[:],
                            lhsT=wv_sb[:, dhp:dhp + 2, eh * P:(eh + 1) * P],
                            rhs=ZT8[:, dhp:dhp + 2, :],
                            start=(dhp == 0), stop=(dhp == DH - 2),
                            perf_mode=DR,
                        )
                    # evict on Vector: Scalar is saturated with Phase D psum
                    # evictions once the split jobs start
                    tmp = att_sb.tile([P, 2 * QT], bf16, tag="tmp")
                    nc.vector.tensor_scalar_mul(tmp[:], a_ps[:],
                                                float(2.0 ** -11))
                    nc.gpsimd.tensor_add(hqT[:, eh, qs], hqT[:, eh, qs], tmp[:])

            scores_j(0)
            scores_j(1)
            zt_ps = tc.alloc_tile_pool(name="zt_ps", bufs=2, space="PSUM", side="right")
            at_ps = tc.alloc_tile_pool(name="at_ps", bufs=2, space="PSUM", side="right")
            ctail_pair(0, zt_ps, at_ps)
            scores_j(2)
            scores_j(3)
            sc_ps.release()
            dn_ps.release()

            # ---- Phase D: logits = hiddenT^T WoT (bias added on host) ----
            nt = (V + VT - 1) // VT
            groups = []
            i = 0
            while i < nt:
                n0 = i * VT
                if i + 1 < nt:
                    groups.append([(n0, min(VT, V - n0)), (n0 + VT, min(VT, V - n0 - VT))])
                    i += 2
                else:
                    groups.append([(n0, min(VT, V - n0))])
                    i += 1

            # jobs: (group, m_lo, m_hi); first KA groups run split so the
            # m0..3 half starts as soon as j0/j1 hidden rows are ready
            jobs = [(g, 0, TQ // 2) for g in groups[:KA]] \
                 + [(g, TQ // 2, TQ) for g in groups[:KA]] \
                 + [(g, 0, TQ) for g in groups[KA:]]

            def load_wo(grp):
                gw = sum(nw for _, nw in grp)
                g0 = grp[0][0]
                wo_c0 = wo_pool.tile([P, DH // 2, 2 * VT], bf16, tag="wo")
                wo_c1 = wo_pool.tile([P, DH // 2, 2 * VT], bf16, tag="wo")
                nc.sync.dma_start(wo_c0[:, :, :gw], woT_r[:, 0:DH // 2, g0:g0 + gw])
                nc.sync.dma_start(wo_c1[:, :, :gw], woT_r[:, DH // 2:DH, g0:g0 + gw])
                return wo_c0, wo_c1

            out_ps = tc.alloc_tile_pool(name="out_ps1", bufs=4, space="PSUM")
            # prefetch two jobs ahead so wo never starves the PE
            wo_q = [load_wo(jobs[0][0]), load_wo(jobs[1][0])]
            for ji, (grp, m_lo, m_hi) in enumerate(jobs):
                gw = sum(nw for _, nw in grp)
                g0 = grp[0][0]
                wo_c0, wo_c1 = wo_q.pop(0)
                if ji + 2 < len(jobs):
                    wo_q.append(load_wo(jobs[ji + 2][0]))
                if ji == 2 * KA:
                    # all split jobs done: attention tail is retired, swap
                    # to the full-width psum pool
                    at_ps.release()
                    zt_ps.release()
                    out_ps.release()
                    out_ps = tc.alloc_tile_pool(name="out_ps2", bufs=8, space="PSUM")
                for m in range(m_lo, m_hi):
                    pss = []
                    for _j in grp:
                        ps_t = out_ps.tile([P, VT], f32, tag="out")
                        pss.append(ps_t)
                    for dh in range(DH):
                        wo_t = wo_c0 if dh < DH // 2 else wo_c1
                        for j, (n0, nw) in enumerate(grp):
                            nc.tensor.matmul(
                                pss[j][:, :nw],
                                lhsT=hqT[:, dh, m * P:(m + 1) * P],
                                rhs=wo_t[:, dh % (DH // 2), j * VT:j * VT + nw],
                                start=(dh == 0), stop=(dh == DH - 1),
                            )
                    lo = out_sb.tile([P, 2 * VT], bf16, tag="lo")
                    for j, (n0, nw) in enumerate(grp):
                        nc.scalar.activation(
                            lo[:, j * VT:j * VT + nw], pss[j][:, :nw],
                            Copy, scale=1.0)
                    nc.sync.dma_start(y_d[m * P:(m + 1) * P, g0:g0 + gw], lo[:, :gw])
                # attention tail interleaves with the first split jobs
                if ji == 0:
                    ctail_pair(2, zt_ps, at_ps)
            out_ps.release()

    nc.compile()
    return nc


def _get_program():
    global _COMPILED
    if _COMPILED is None:
        _COMPILED = _build_program()
    return _COMPILED


def _core_rows(hh):
    tiles = TILES_A if hh == 0 else TILES_B
    return np.concatenate([np.arange(t * QT, (t + 1) * QT) for t in tiles])


def kernel(x, embed, pe, Wq, Wk, Wv, Wo, bo):
    import ml_dtypes
    from concourse.bass_utils import run_bass_kernel_spmd

    bf16 = ml_dtypes.bfloat16
    fp8 = ml_dtypes.float8_e4m3fn
    x = np.asarray(x).astype(np.int32)
    embed = np.asarray(embed, dtype=np.float32)
    pe = np.asarray(pe, dtype=np.float32)
    Wq = np.asarray(Wq, dtype=np.float32)
    Wk = np.asarray(Wk, dtype=np.float32)

    h_all = (embed[x.reshape(-1)].reshape(B, C, D) + pe[None, :C, :]) * SH
    h8_all = np.clip(h_all, -240.0, 240.0).astype(fp8)
    h_all = h_all.astype(bf16)
    wm8 = np.ascontiguousarray(
        np.clip((Wq.T @ Wk) * SW, -240.0, 240.0).astype(fp8))
    wv8 = np.ascontiguousarray(
        np.clip(np.asarray(Wv, dtype=np.float32).T * SW, -240.0, 240.0).astype(fp8))
    woT = np.ascontiguousarray(
        (np.asarray(Wo, dtype=np.float32).T * (1.0 / SH)).astype(bf16))
    bo_np = np.asarray(bo, dtype=np.float32).reshape(V)

    nc = _get_program()

    rows = {hh: _core_rows(hh) for hh in range(2)}
    in_maps = []
    for c in range(N_CORES):
        b, hh = c // 2, c % 2
        tiles = TILES_A if hh == 0 else TILES_B
        # pair-masks in program order: j-major, masked pairs only
        blocks = []
        for j in range(NQT):
            for kbp in range(MUL_FROM[j], BOUND[j], 2):
                qpos = tiles[j] * QT + np.arange(QT)
                kpos0 = kbp * P + np.arange(P)
                kpos1 = (kbp + 1) * P + np.arange(P)
                m0 = (qpos[None, :] >= kpos0[:, None]).astype(fp8)
                m1 = (qpos[None, :] >= kpos1[:, None]).astype(fp8)
                blocks.append(np.stack([m0, m1], axis=1))  # [P, 2, QT]
        maskarr = np.ascontiguousarray(np.stack(blocks, axis=1))  # [P, NMASKP, 2, QT]
        assert maskarr.shape == (P, NMASKP, 2, QT)
        hq = h_all[b][rows[hh]]          # [NQ, D] bf16
        hq8 = h8_all[b][rows[hh]]        # [NQ, D] fp8
        in_maps.append({
            "hqT8": np.ascontiguousarray(hq8.T),
            "wm8": wm8,
            "hqT": np.ascontiguousarray(hq.T),
            "hkvT8": np.ascontiguousarray(h8_all[b].T),
            "hkv8": h8_all[b],
            "wv": wv8,
            "woT": woT,
            "mask": maskarr,
        })

    global _last_in_maps
    _last_in_maps = in_maps
    res = run_bass_kernel_spmd(nc, in_maps, core_ids=list(range(N_CORES)))

    out = np.empty((B, C, V), dtype=np.float32)
    for c in range(N_CORES):
        b, hh = c // 2, c % 2
        out[b, rows[hh], :] = res.results[c]["y"].astype(np.float32)
    if np.any(bo_np):
        out += bo_np[None, None, :]
    return out
